# revision 19
# baseline (speedup 1.0000x reference)
"""Trainium2 kernel for the FEM kinematic (strain) layer.

Reference computation:
    disp = inputs[:, elem_nodes]                      # [B, E, 8, 2]
    dd   = einsum('egkl,bekn->begnl', shpdx, disp)    # [B, E, 9, 2, 2]
    out  = stack([dd[...,0,0], dd[...,1,1],
                  0.5*(dd[...,0,1] + dd[...,1,0])])   # [B, E*9, 3]

Sharding: elements split across 8 NeuronCores.  The host resolves the
element->node indirection and ships per-core element-major blocks in
bf16: the two shpdx l-planes (S0, S1), their sum A, and displacement
planes u, v, w=u+v laid out (b, c, k) per partition.  The device
computes, per element and gauss point,
    e_xx = sum_k S0*u,  e_yy = sum_k S1*v,
    e_xy = 0.5*(sum_k A*w - e_xx - e_yy)
with three batched bf16 multiplies (DVE 2x packed mode), segmented
k-sums as packed add-trees (8 -> 4 -> 2 -> 1) on DVE (xx/yy) and Pool
(xy path + subtracts), and the 0.5-scale on Activation.  Output is
written bf16 and widened on the host.
"""

import sys
import numpy as np

sys.path.insert(0, "/opt/trn_rl_repo")

import ml_dtypes

import concourse.bass as bass
import concourse.bacc as bacc
import concourse.mybir as mybir
import concourse.tile as tile
from concourse.bass_utils import run_bass_kernel_spmd

BF16 = ml_dtypes.bfloat16

B = 4
N_NODES = 1_000_000
N_ELEM = 500_000
N_GP = 9
N_EN = 8
N_CORES = 8

E_CORE = N_ELEM // N_CORES            # 62500 elements per core
P = 128                               # SBUF partitions
C = 16                                # elements per partition per chunk
CHUNK = P * C                         # 2048 elements per chunk
N_CHUNKS = -(-E_CORE // CHUNK)        # 31
E_PAD = N_CHUNKS * CHUNK              # 63488 (988 pad elements)

_compiled = None


def _build_program():
    nc = bacc.Bacc("TRN2", target_bir_lowering=False, debug=False)
    bf = mybir.dt.bfloat16

    # shape-function planes, per element (g, k)
    s0_d = nc.dram_tensor("s0", [E_PAD, 72], bf, kind="ExternalInput").ap()
    s1_d = nc.dram_tensor("s1", [E_PAD, 72], bf, kind="ExternalInput").ap()
    a_d = nc.dram_tensor("a", [E_PAD, 72], bf, kind="ExternalInput").ap()
    # displacement planes, per (chunk, partition): (b, c, k)
    du_d = nc.dram_tensor("du", [N_CHUNKS * P, B * C * 8], bf, kind="ExternalInput").ap()
    dv_d = nc.dram_tensor("dv", [N_CHUNKS * P, B * C * 8], bf, kind="ExternalInput").ap()
    dw_d = nc.dram_tensor("dw", [N_CHUNKS * P, B * C * 8], bf, kind="ExternalInput").ap()
    # [128, 16] bf16 ones-blockdiag: W[p, m] = 1 if p // 8 == m
    w_d = nc.dram_tensor("wones", [128, 16], bf, kind="ExternalInput").ap()
    # [B, E_PAD*9, 3] bf16
    o_d = nc.dram_tensor("out", [B, E_PAD * 9, 3], bf, kind="ExternalOutput").ap()

    s0_v = s0_d.rearrange("(n p c) f -> n p (c f)", p=P, c=C)
    s1_v = s1_d.rearrange("(n p c) f -> n p (c f)", p=P, c=C)
    a_v = a_d.rearrange("(n p c) f -> n p (c f)", p=P, c=C)
    du_v = du_d.rearrange("(n p) f -> n p f", p=P)
    dv_v = dv_d.rearrange("(n p) f -> n p f", p=P)
    dw_v = dw_d.rearrange("(n p) f -> n p f", p=P)
    o_v = o_d.rearrange("b (n p x) three -> b n p (x three)", p=P, x=C * 9)

    add = mybir.AluOpType.add
    sub = mybir.AluOpType.subtract
    mult = mybir.AluOpType.mult

    with tile.TileContext(nc) as tc:
        with (
            tc.tile_pool(name="io", bufs=3) as io_pool,
            tc.tile_pool(name="tmp", bufs=2) as tmp_pool,
            tc.tile_pool(name="wpool", bufs=1) as w_pool,
            tc.tile_pool(name="ps", bufs=2, space="PSUM") as psum_pool,
        ):
            Wones = w_pool.tile([128, 16], bf, tag="W")
            nc.sync.dma_start(out=Wones[:], in_=w_d)
            for i in range(N_CHUNKS):
                S0 = io_pool.tile([P, C * 72], bf, tag="S0")
                S1 = io_pool.tile([P, C * 72], bf, tag="S1")
                A = io_pool.tile([P, C * 72], bf, tag="A")
                Du = io_pool.tile([P, B * C * 8], bf, tag="Du")
                Dv = io_pool.tile([P, B * C * 8], bf, tag="Dv")
                Dw = io_pool.tile([P, B * C * 8], bf, tag="Dw")
                nc.sync.dma_start(out=S0[:], in_=s0_v[i])
                nc.sync.dma_start(out=S1[:], in_=s1_v[i])
                nc.sync.dma_start(out=A[:], in_=a_v[i])
                nc.sync.dma_start(out=Du[:], in_=du_v[i])
                nc.sync.dma_start(out=Dv[:], in_=dv_v[i])
                nc.sync.dma_start(out=Dw[:], in_=dw_v[i])

                O = io_pool.tile([P, B * C * 27], bf, tag="O")
                # (bc, t, g) view of the (b, c, g, t) staging layout
                Oxy = O[:].rearrange("p (b c g t) -> p (b c) t g", b=B, c=C, g=9)

                def splane(t):
                    r = t[:].rearrange("p (c g k) -> p c g k", c=C, g=9)
                    return r[:, None, :, :, :].to_broadcast([P, B, C, 9, 8])

                def dplane(t):
                    r = t[:].rearrange("p (b c k) -> p b c k", b=B, c=C)
                    return r[:, :, :, None, :].to_broadcast([P, B, C, 9, 8])

                # ---- products ------------------------------------------
                # T01[l, b, c, g, k]: l=0 -> S0*u (xx), l=1 -> S1*v (yy)
                T01 = tmp_pool.tile([P, 2 * B * C * 72], bf, tag="T01")
                T01v = T01[:].rearrange(
                    "p (l b c g k) -> p l b c g k", l=2, b=B, c=C, g=9
                )
                T2 = tmp_pool.tile([P, B * C * 72], bf, tag="T2")
                T2v = T2[:].rearrange("p (b c g k) -> p b c g k", b=B, c=C, g=9)
                nc.vector.tensor_tensor(
                    out=T01v[:, 0], in0=splane(S0), in1=dplane(Du), op=mult
                )
                nc.vector.tensor_tensor(
                    out=T01v[:, 1], in0=splane(S1), in1=dplane(Dv), op=mult
                )
                nc.vector.tensor_tensor(
                    out=T2v, in0=splane(A), in1=dplane(Dw), op=mult
                )

                # ---- k-sum add-trees ------------------------------------
                # xx+yy tree on DVE, fused over (l b c): [*, g, 8]->4->2->O
                T01f = T01[:].rearrange("p (q g k) -> p q g k", g=9, k=8)
                Q4 = tmp_pool.tile([P, 2 * B * C * 36], bf, tag="Q4")
                Q4v = Q4[:].rearrange("p (q g k) -> p q g k", g=9, k=4)
                nc.vector.tensor_tensor(
                    out=Q4v, in0=T01f[:, :, :, 0:4], in1=T01f[:, :, :, 4:8],
                    op=add,
                )
                Q2 = tmp_pool.tile([P, 2 * B * C * 18], bf, tag="Q2")
                Q2v = Q2[:].rearrange("p (q g k) -> p q g k", g=9, k=2)
                nc.gpsimd.tensor_tensor(
                    out=Q2v, in0=Q4v[:, :, :, 0:2], in1=Q4v[:, :, :, 2:4],
                    op=add,
                )
                # stage3 fused: dims (l, bc, g) -> O[t=l]; alternates to
                # Pool on some chunks to balance engine load
                Q2t = Q2[:].rearrange(
                    "p (l bc g k) -> p l bc g k", l=2, g=9, k=2
                )
                nc.gpsimd.tensor_tensor(
                    out=Oxy[:, :, 0:2, :].rearrange("p bc t g -> p t bc g"),
                    in0=Q2t[:, :, :, :, 0], in1=Q2t[:, :, :, :, 1],
                    op=add,
                )

                # xy k-sums on PE: block-transpose puts k on partitions,
                # ones-blockdiag matmul sums k-groups, Act evacuates PSUM,
                # transpose-back restores element-partition layout.
                F2 = B * C * 72                       # 4608
                NB = F2 // 128                        # 36 blocks
                T2t = tmp_pool.tile([P, F2], bf, tag="T2t")
                nc.scalar.dma_start_transpose(
                    out=T2t[:].rearrange("p (t q) -> p t q", q=128),
                    in_=T2[:],
                )
                Y = tmp_pool.tile([16, F2], bf, tag="Y")
                NS = 3
                SL = F2 // NS                         # 1536 (3 PSUM banks)
                for s in range(NS):
                    ps = psum_pool.tile([16, SL], mybir.dt.float32, tag="ps")
                    for j in range(SL // 512):
                        nc.tensor.matmul(
                            ps[:, j * 512:(j + 1) * 512], Wones[:],
                            T2t[:, s * SL + j * 512:s * SL + (j + 1) * 512],
                            start=True, stop=True,
                        )
                    nc.scalar.activation(
                        out=Y[:, s * SL:(s + 1) * SL], in_=ps[:],
                        func=mybir.ActivationFunctionType.Copy,
                    )
                X2 = tmp_pool.tile([P, B * C * 9], bf, tag="X2")
                nc.scalar.dma_start_transpose(
                    out=X2[:].rearrange("p (t r) -> p t r", r=16),
                    in_=Y[:],
                )
                X2v = X2[:].rearrange("p (bc g) -> p bc g", g=9)
                nc.gpsimd.tensor_tensor(
                    out=X2v, in0=X2v, in1=Oxy[:, :, 0, :], op=sub
                )
                nc.gpsimd.tensor_tensor(
                    out=X2v, in0=X2v, in1=Oxy[:, :, 1, :], op=sub
                )
                # e_xy = 0.5 * X2  (Activation)
                nc.scalar.activation(
                    out=Oxy[:, :, 2, :], in_=X2v,
                    func=mybir.ActivationFunctionType.Copy, scale=0.5,
                )

                for b in range(B):
                    nc.scalar.dma_start(
                        out=o_v[b, i],
                        in_=O[:, b * C * 27:(b + 1) * C * 27],
                    )

    nc.compile()
    return nc


def _get_program():
    global _compiled
    if _compiled is None:
        _compiled = _build_program()
    return _compiled


def kernel(inputs, shpdx, elem_nodes, _want_trace=False):
    nc = _get_program()

    in_maps = []
    for c in range(N_CORES):
        sl = slice(c * E_CORE, (c + 1) * E_CORE)
        sp = shpdx[sl]                                        # [E, 9, 8, 2] f32

        def pad72(x):
            out = np.zeros((E_PAD, 72), BF16)
            out[:E_CORE] = x.reshape(E_CORE, 72).astype(BF16)
            return out

        s0 = pad72(np.ascontiguousarray(sp[..., 0]))          # (g, k)
        s1 = pad72(np.ascontiguousarray(sp[..., 1]))
        a = pad72(sp.sum(axis=3))

        en = elem_nodes[sl]                                   # [E, 8]
        disp = inputs[:, en]                                  # [B, E, 8, 2]

        # displacement planes in (chunk, partition, b, c, k) layout
        def dplane(x):                                        # x: [B, E, 8] f32
            xp = np.zeros((B, E_PAD, 8), np.float32)
            xp[:, :E_CORE] = x
            # [B, n, p, c, k] -> [n, p, b, c, k]
            xp = xp.reshape(B, N_CHUNKS, P, C, 8).transpose(1, 2, 0, 3, 4)
            return np.ascontiguousarray(xp).reshape(N_CHUNKS * P, B * C * 8).astype(BF16)

        u = disp[..., 0]
        v = disp[..., 1]
        wones = np.zeros((128, 16), BF16)
        wones[np.arange(128), np.arange(128) // 8] = 1.0
        in_maps.append({
            "s0": s0, "s1": s1, "a": a,
            "du": dplane(u), "dv": dplane(v), "dw": dplane(u + v),
            "wones": wones,
        })

    core_ids = list(range(N_CORES))
    res = run_bass_kernel_spmd(nc, in_maps, core_ids, trace=_want_trace)

    outs = []
    for c in range(N_CORES):
        o = res.results[c]["out"]                             # [B, E_PAD*9, 3] bf16
        outs.append(o[:, :E_CORE * 9, :].astype(np.float32))
    full = np.concatenate(outs, axis=1)                       # [B, N_ELEM*9, 3]
    if _want_trace:
        return full, res
    return full
